# revision 11
# baseline (speedup 1.0000x reference)
"""Trainium2 Bass kernel for DifferentiableRankIntegration (oct-packed G=16).

Math (per query row i, B=1024):
  sig[k,j] = sigmoid((s[i,k] - s[i,j]) / tau),  tau = 0.1
  Sp[j] = sum_k pos[i,k]*sig[k,j],  Sn[j] = sum_k neg[i,k]*sig[k,j]
  rank[j] = 1 + Sn[j] if pos[i,j] else 1 + Sp[j]
  out[i,j] = (K+1) * (w_v/(K+rank_v) + w_l/(K+rank_l)),  K = 60

Grid factorization with G=16 points on [-5.2, 5.2]:
  pass-1: F[m] = sum_k m_k sigmoid(10(s_k - g_m))          (ACT + PE)
  filter: W = A @ F, A a dense [G,G] matrix fitted offline by two-sided
          ridge LSQ so that sum_mu (A F)_mu sigmoid((g_mu - x)/wrec)
          reproduces a single sigmoid step at any center (rel err of the
          full output ~3.1e-3, budget 2e-2)
  pass-2: f(s_j) ~= sum_mu W_mu sigmoid((g_mu - s_j)/wrec)

Layout: NT=8 rows ("oct") share each [128, 2048] ACT instruction; per
core 16 octs cover the 128 local rows.
  pass-1 E: [p = k mod 128, (Q-contig: q, c, t, m)], diff built by one
          DVE op with broadcast access patterns from resident bf16 sT10c
          (4-way split DMA so oct 0 starts early).
  pass-2 phi: [p = (t, m), (q, j)], input is host-replicated bf16 bc2q
          DMA'd per oct, alternating the SP/ACT HWDGE queues.
  pass-1 PE: per (q, c) one matmul lhsT=E-block [128, 128=(t,m)] x
          rhs=wint8 [128, 16=(t',mask)] -> Fps[(t,m), (q,t',mask)]; the
          t!=t' blocks are garbage, zeroed by folding a 0/1 block-mask
          into the PSUM->SBUF eviction (tensor_mul instead of copy).
  filter PE: ONE matmul lhsT=TdAll (block-diagonal A.T) for both q.
  pass-2 PE: per (q, c) lhsT=phi-block [128, 128 j] x rhs=W [128, 16]
          -> acc[j, (q,c,t,mask)]: j on partitions, so eviction is ONE
          [128, 256] DVE op per oct (also folds in the +K+1).
Finals run in the j-partition layout as quarter-batches interleaved with
the oct loop (pos-select via copy_predicated with a uint8 mask); host
pre-scales w by K+1 and unshards the [jp, (Q,c,t)] output layout.
"""

import sys

import numpy as np

if "/opt/trn_rl_repo" not in sys.path:
    sys.path.append("/opt/trn_rl_repo")

B = 1024
NCORES = 8
ROWS = B // NCORES  # 128 rows per core
P = 128
NCHUNK = B // P  # 8
TAU = 0.1
K = 60.0

G = 16
NT = 8  # rows per quad
NQUAD = ROWS // NT  # 32
LO, HI = -5.2, 5.2
H = (HI - LO) / (G - 1)
WREC = 0.65 * H


def _sigmoid(x):
    return 1.0 / (1.0 + np.exp(-np.clip(x, -80, 80)))


def _fit_A(lam=1e-7, npts=1601):
    g = np.linspace(LO, HI, G)
    xs = np.linspace(LO - 0.5, HI + 0.5, npts)
    ss = xs
    Phi = _sigmoid((g[None, :] - xs[:, None]) / WREC)  # [X, G]
    Bm = _sigmoid((ss[None, :] - g[:, None]) / TAU)  # [G, S]
    T = _sigmoid((ss[None, :] - xs[:, None]) / TAU)  # [X, S]
    wx = np.exp(-0.25 * xs**2) + 1e-3
    ws = wx
    Phi_w = Phi * wx[:, None]
    T_w = T * wx[:, None] * ws[None, :]
    Bm_w = Bm * ws[None, :]
    PtP = Phi_w.T @ Phi_w + lam * np.trace(Phi_w.T @ Phi_w) / G * np.eye(G)
    BBt = Bm_w @ Bm_w.T + lam * np.trace(Bm_w @ Bm_w.T) / G * np.eye(G)
    A = np.linalg.solve(PtP, Phi_w.T @ T_w @ Bm_w.T) @ np.linalg.inv(BBt)
    return g, A


def _build_consts():
    g, A = _fit_A()
    grid10 = np.tile((10.0 * g)[None, :], (P, 1)).astype(np.float32)
    # TdAll [128, 128]: block-diagonal with A.T in every t-block.
    tdsel = np.zeros((P, P), np.float64)
    for t in range(NT):
        tdsel[G * t : G * t + G, G * t : G * t + G] = A.T
    biasg = (g[np.arange(P) % G] / WREC).reshape(P, 1)
    # blockmask[(t, m1), (t', mask)] = 1 if t == t' else 0
    bmask = np.zeros((P, 4 * NT), np.float32)
    for t in range(NT):
        bmask[G * t : G * t + G, 2 * t : 2 * t + 2] = 1.0
        bmask[G * t : G * t + G, 2 * NT + 2 * t : 2 * NT + 2 * t + 2] = 1.0
    return (
        grid10,
        tdsel.astype(np.float32),
        biasg.astype(np.float32),
        bmask,
    )


GRID10, TDSEL, BIASG, BMASK = _build_consts()


def _build_bass():
    import concourse.bacc as bacc
    import concourse.mybir as mybir
    from concourse.tile import TileContext

    f32 = mybir.dt.float32
    bf16 = mybir.dt.bfloat16

    nc = bacc.Bacc()

    # Per-core inputs (host pre-sharded / pre-transposed):
    # sT10c[p, Q*128 + (q*8+c)*NT + t] = 10 * s_q[NT*Q+t, c*128+p]
    sT10c = nc.declare_dram_parameter("sT10c", [P, 2 * B], bf16, isOutput=False)
    # bc2q[32t+m, Q*2048 + q*1024 + j] = s_q[4Q+t, j]  (pass-2 input, replicated)
    bc2q = nc.declare_dram_parameter("bc2q", [P, NQUAD * 2 * B], bf16, isOutput=False)
    # wint8[p, (Q*8+c)*8 + t*2 + mask] = mask[4Q+t, c*128+p] (0=pos, 1=neg)
    wint8 = nc.declare_dram_parameter("wint8", [P, 2 * B], bf16, isOutput=False)
    grid10 = nc.declare_dram_parameter("grid10", [P, G], f32, isOutput=False)
    tdsel = nc.declare_dram_parameter("tdsel", [P, P], f32, isOutput=False)
    bmask = nc.declare_dram_parameter("bmask", [P, 4 * NT], f32, isOutput=False)
    biasg = nc.declare_dram_parameter("biasg", [P, 1], f32, isOutput=False)
    # finals inputs in j-partition layout: [jp, (Q*8+c)*4 + t]
    posj = nc.declare_dram_parameter("posj", [P, B], mybir.dt.uint8, isOutput=False)
    wvj = nc.declare_dram_parameter("wvj", [P, B], f32, isOutput=False)
    wlj = nc.declare_dram_parameter("wlj", [P, B], f32, isOutput=False)
    out = nc.declare_dram_parameter("out", [P, B], f32, isOutput=True)

    with TileContext(nc) as tc:
        with (
            tc.tile_pool(name="const", bufs=1) as cpool,
            tc.tile_pool(name="diff", bufs=4) as dpool,
            tc.tile_pool(name="esig", bufs=4) as epool,
            tc.tile_pool(name="bcast", bufs=4) as bpool,
            tc.tile_pool(name="phi", bufs=4) as phpool,
            tc.tile_pool(name="fsb", bufs=3) as fbpool,
            tc.tile_pool(name="fin", bufs=1) as fpool,
            tc.tile_pool(name="psum_f", bufs=2, space="PSUM") as ppool_f,
            tc.tile_pool(name="psum_g", bufs=2, space="PSUM") as ppool_g,
            tc.tile_pool(name="psum_a", bufs=2, space="PSUM") as ppool_a,
        ):
            # --- load resident inputs ---
            sT_t = cpool.tile([P, 2 * B], bf16, tag="sT")
            wint_t = cpool.tile([P, 2 * B], bf16, tag="wint")
            grid10_t = cpool.tile([P, G], f32, tag="grid10")
            tdsel_t = cpool.tile([P, P], f32, tag="tdsel")
            bmask_t = cpool.tile([P, 4 * NT], f32, tag="bmask")
            biasg_t = cpool.tile([P, 1], f32, tag="biasg")
            for i4 in range(4):
                nc.sync.dma_start(
                    out=sT_t[:, i4 * 512 : (i4 + 1) * 512],
                    in_=sT10c[:, i4 * 512 : (i4 + 1) * 512],
                )
            nc.sync.dma_start(out=wint_t[:], in_=wint8[:])
            nc.sync.dma_start(out=grid10_t[:], in_=grid10[:])
            nc.sync.dma_start(out=tdsel_t[:], in_=tdsel[:])
            nc.sync.dma_start(out=bmask_t[:], in_=bmask[:])
            nc.sync.dma_start(out=biasg_t[:], in_=biasg[:])

            pos_t = fpool.tile([P, B], mybir.dt.uint8, tag="pos")
            wv_t = fpool.tile([P, B], f32, tag="wv")
            wl_t = fpool.tile([P, B], f32, tag="wl")
            nc.sync.dma_start(out=pos_t[:], in_=posj[:])
            nc.sync.dma_start(out=wv_t[:], in_=wvj[:])
            nc.sync.dma_start(out=wl_t[:], in_=wlj[:])

            spc = fpool.tile([P, NQUAD * 32 * NT], f32, tag="spc")

            res = fpool.tile([P, B], f32, tag="res")
            t_v = fpool.tile([P, B], f32, tag="t_v")

            def _emit_finals(o0, o1):
                # finals for octs [o0, o1): all tiles sliced on the (Q,...) axis
                nq = o1 - o0
                aw_ = 32 * NT
                spc_r = spc[:, o0 * aw_ : o1 * aw_].rearrange(
                    "p (Q q c t m) -> p q m Q c t", Q=nq, q=2, c=NCHUNK, t=NT, m=2
                )
                fb = o0 * NCHUNK * NT
                fe = o1 * NCHUNK * NT
                for q, (w_t, dst) in enumerate(((wv_t, None), (wl_t, res))):
                    sp = spc_r[:, q, 0]
                    sn = spc_r[:, q, 1]
                    d1 = fpool.tile([P, B], f32, tag=f"d1_{q}")
                    d1s = d1[:, fb:fe]
                    d1r = d1s.rearrange("p (Q c t) -> p Q c t", Q=nq, c=NCHUNK, t=NT)
                    pos_r = pos_t[:, fb:fe].rearrange(
                        "p (Q c t) -> p Q c t", Q=nq, c=NCHUNK, t=NT
                    )
                    # spc already holds count + K + 1; select pos ? sn : sp
                    nc.vector.tensor_copy(d1r, sp)
                    nc.vector.copy_predicated(d1r, pos_r, sn)
                    nc.vector.reciprocal(d1s, d1s)
                    if dst is None:
                        nc.vector.tensor_mul(t_v[:, fb:fe], w_t[:, fb:fe], d1s)
                    else:
                        nc.vector.tensor_mul(d1s, w_t[:, fb:fe], d1s)
                        nc.vector.tensor_add(res[:, fb:fe], t_v[:, fb:fe], d1s)
                nc.sync.dma_start(out=out[:, fb:fe], in_=res[:, fb:fe])

            # Sp/Sn destination tiles (filled row by row)
            for Q in range(NQUAD):
                # pass-1 diff: [p, (q, c, t, m)] = 10*s - 10*g via one DVE op
                diff_t = dpool.tile([P, 2 * B], bf16, tag="diff")
                nx = 2 * NCHUNK * NT
                scols = sT_t[:, Q * nx : (Q + 1) * nx].broadcast_to((P, nx, G))
                gbc = grid10_t[:, None, :].broadcast_to((P, nx, G))
                nc.vector.tensor_sub(
                    diff_t[:].rearrange("p (x m) -> p x m", x=nx), scols, gbc
                )
                esig = epool.tile([P, 2 * B], bf16, tag="esig")
                nc.scalar.activation(
                    out=esig[:],
                    in_=diff_t[:],
                    func=mybir.ActivationFunctionType.Sigmoid,
                    bias=0.0,
                    scale=1.0,
                )
                # pass-2 input: bc2[(t, m), (q, j)] = s_q[4Q+t, j] (DMA'd replicated)
                bc2 = bpool.tile([P, 2 * B], bf16, tag="bc")
                dmaq = nc.sync if Q % 2 == 0 else nc.scalar
                dmaq.dma_start(
                    out=bc2[:], in_=bc2q[:, Q * 2 * B : (Q + 1) * 2 * B]
                )
                phi = phpool.tile([P, 2 * B], bf16, tag="phi")
                nc.scalar.activation(
                    out=phi[:],
                    in_=bc2[:],
                    func=mybir.ActivationFunctionType.Sigmoid,
                    bias=biasg_t[:, 0:1],
                    scale=-1.0 / WREC,
                )
                # pass-1 contraction: Fps[(t,m), (q, t', mask)]
                w2 = 2 * NT
                fps = ppool_f.tile([P, 512], f32, tag="fps")
                for q in range(2):
                    for c in range(NCHUNK):
                        nc.tensor.matmul(
                            out=fps[:, w2 * q : w2 * q + w2],
                            lhsT=esig[:, (q * 8 + c) * P : (q * 8 + c + 1) * P],
                            rhs=wint_t[:, (Q * 8 + c) * w2 : (Q * 8 + c + 1) * w2],
                            start=(c == 0),
                            stop=(c == NCHUNK - 1),
                        )
                fsb = fbpool.tile([P, 2 * w2], f32, tag="fsb")
                # masked copy zeroes the t!=t' garbage blocks of F
                nc.vector.tensor_mul(fsb[:], fps[:, 0 : 2 * w2], bmask_t[:])
                # filter: W = A @ F, one block-diagonal matmul for both q
                f2ps = ppool_g.tile([P, 512], f32, tag="f2ps")
                nc.tensor.matmul(
                    out=f2ps[:, 0 : 2 * w2],
                    lhsT=tdsel_t[:],
                    rhs=fsb[:],
                    start=True,
                    stop=True,
                )
                f2sb = fbpool.tile([P, 2 * w2], bf16, tag="f2sb")
                nc.vector.tensor_copy(f2sb[:], f2ps[:, 0 : 2 * w2])
                # pass-2: acc[j, (q, c, t, mask)]
                aw = 16 * w2
                acc = ppool_a.tile([P, 512], f32, tag="acc")
                for q in range(2):
                    for c in range(NCHUNK):
                        nc.tensor.matmul(
                            out=acc[:, (q * 8 + c) * w2 : (q * 8 + c + 1) * w2],
                            lhsT=phi[:, q * B + c * P : q * B + (c + 1) * P],
                            rhs=f2sb[:, w2 * q : w2 * q + w2],
                            start=True,
                            stop=True,
                        )
                nc.vector.tensor_scalar_add(spc[:, Q * aw : (Q + 1) * aw], acc[:, 0:aw], K + 1.0)
                if Q in (NQUAD // 4, NQUAD // 2, 3 * NQUAD // 4):
                    _emit_finals(Q - NQUAD // 4, Q)

            # --- finals, batched in j-partition layout ---
            _emit_finals(3 * NQUAD // 4, NQUAD)
            # === end of timed body ===

    nc.compile()
    return nc


_NC_CACHE = None


def _get_nc():
    global _NC_CACHE
    if _NC_CACHE is None:
        _NC_CACHE = _build_bass()
    return _NC_CACHE


def _prep_core_inputs(s_v, s_l, pos_f, neg_f, w_v, w_l, core):
    import ml_dtypes

    lo, hi = core * ROWS, (core + 1) * ROWS
    sv = np.ascontiguousarray(s_v[lo:hi]).astype(np.float32)
    sl = np.ascontiguousarray(s_l[lo:hi]).astype(np.float32)

    # sT10c[p, Q, q, c, t] = 10*s_q[NT*Q+t, c*128+p]
    arr = np.stack([sv, sl])  # [q, r, j]
    a5 = arr.reshape(2, NQUAD, NT, NCHUNK, P)  # [q, Q, t, c, p]
    sT10c = 10.0 * a5.transpose(4, 1, 0, 3, 2).reshape(P, 2 * B)

    # bc2q[32t+m, Q, q, j] = s_q[4Q+t, j]
    aq = arr.reshape(2, NQUAD, NT, B).transpose(2, 1, 0, 3)  # [t, Q, q, j]
    bc2q = np.broadcast_to(
        aq[:, None, :, :, :], (NT, G, NQUAD, 2, B)
    ).reshape(P, NQUAD * 2 * B)

    # wint8[p, Q, c, t, mask]
    masks = np.stack([pos_f[lo:hi], neg_f[lo:hi]])  # [mask, r, j]
    m5 = masks.reshape(2, NQUAD, NT, NCHUNK, P)  # [mask, Q, t, c, p]
    wint8 = m5.transpose(4, 1, 3, 2, 0).reshape(P, 2 * B)

    def jlay(x):
        # [jp, (Q*8+c)*4 + t] = x[4Q+t, c*128+jp]
        x4 = np.asarray(x, np.float32).reshape(NQUAD, NT, NCHUNK, P)
        return np.ascontiguousarray(x4.transpose(3, 0, 2, 1).reshape(P, B))

    return {
        "sT10c": np.ascontiguousarray(sT10c).astype(ml_dtypes.bfloat16),
        "bc2q": np.ascontiguousarray(bc2q).astype(ml_dtypes.bfloat16),
        "wint8": np.ascontiguousarray(wint8).astype(ml_dtypes.bfloat16),
        "grid10": GRID10,
        "tdsel": TDSEL,
        "bmask": BMASK,
        "biasg": BIASG,
        "posj": jlay(pos_f[lo:hi]).astype(np.uint8),
        "wvj": jlay((K + 1.0) * w_v[lo:hi]),
        "wlj": jlay((K + 1.0) * w_l[lo:hi]),
    }


def _unshard_core_out(o):
    # o[jp, (Q*8+c)*4 + t] -> [r = 4Q+t, j = c*128+jp]
    o4 = o.reshape(P, NQUAD, NCHUNK, NT)
    return o4.transpose(1, 3, 2, 0).reshape(ROWS, B)


def _run(in_maps, trace=False):
    from concourse.bass_utils import run_bass_kernel_spmd

    nc = _get_nc()
    return run_bass_kernel_spmd(nc, in_maps, core_ids=list(range(NCORES)), trace=trace)


def kernel(s_v, s_l, pos_mask, neg_mask, w_v, w_l, _trace=False):
    pos_f = pos_mask.astype(np.float32)
    neg_f = neg_mask.astype(np.float32)
    in_maps = [
        _prep_core_inputs(s_v, s_l, pos_f, neg_f, w_v, w_l, core)
        for core in range(NCORES)
    ]
    res = _run(in_maps, trace=_trace)
    outs = [
        _unshard_core_out(np.asarray(res.results[i]["out"], np.float32))
        for i in range(NCORES)
    ]
    full = np.concatenate(outs, axis=0).astype(np.float32)
    if _trace:
        return full, res
    return full
